# revision 3
# baseline (speedup 1.0000x reference)
"""Trainium2 Bass kernel for nn_KANSplineLayer.

Computes, for x:(8192,2048) f32, base_weight:(2048,2048) f32,
grid:(2048,2048,8) f32:

    base_out   = x @ base_weight.T
    basis      = exp(-(x - grid.mean())**2)
    spline_out = basis @ grid.sum(-1)
    out        = base_out + spline_out          # (8192, 2048) f32

Sharding: 8 cores as 2 batch-groups x 4 out-feature groups.
Each core computes a (4096, 512) tile of the output.
  - inputs are pre-cast to bf16 and laid out on the host so that the
    contraction (in-feature) dim lands on SBUF partitions.
  - the scalar grid.mean() needs the *full* grid; each core reduces its
    own grid shard and the partial sums are combined with a tiny
    AllReduce (with a local-mean fallback).
Device schedule (per core):
  pass 0: stream grid shard, tree-add over the G=8 axis -> G matrix,
          accumulate scalar partials, AllReduce -> gm  (vector/gpsimd)
  pass 1: base matmuls x @ W^T into SBUF (tensor engine, overlaps pass 0)
  pass 2: basis = exp(-(x-gm)^2) (vector+scalar), spline matmuls, add
          base, DMA out.
"""

import numpy as np
import ml_dtypes

import concourse.bass as bass
import concourse.mybir as mybir
import concourse.tile as tile
from concourse import bacc, bass_isa
from concourse.bass_utils import run_bass_kernel_spmd

P = 128            # SBUF partitions
IN_F = 2048
OUT_F = 2048
GG = 8             # grid last dim (grid_size + spline_order)
BATCH = 8192
R = 2              # batch groups
C = 4              # out-feature groups
N_CORES = 8
B_SH = BATCH // R      # 4096 batch rows per core
O_SH = OUT_F // C      # 512 out features per core
KO = IN_F // P         # 16 contraction chunks
NBT = B_SH // P        # 32 batch tiles per core
USE_COLLECTIVE = True

BF16 = ml_dtypes.bfloat16

_cached_nc = None


def _build_nc():
    nc = bacc.Bacc(
        "TRN2", target_bir_lowering=False, debug=False, num_devices=N_CORES
    )
    f32 = mybir.dt.float32
    bf16 = mybir.dt.bfloat16
    add = mybir.AluOpType.add

    # Layouts: partition dim first, contraction (in-features) split as
    # (ko, p) so lhsT/rhs matmul operands are direct slices.
    x_in = nc.dram_tensor("xt", [P, NBT, KO, P], bf16, kind="ExternalInput")
    w_in = nc.dram_tensor("wt", [P, KO, O_SH], bf16, kind="ExternalInput")
    g_in = nc.dram_tensor("grid", [P, KO, GG, O_SH], bf16, kind="ExternalInput")
    out = nc.dram_tensor("out", [B_SH, O_SH], f32, kind="ExternalOutput")

    with tile.TileContext(nc) as tc:
        with (
            tc.tile_pool(name="const", bufs=1) as const_pool,
            tc.tile_pool(name="res", bufs=1) as res_pool,
            tc.tile_pool(name="gridp", bufs=2) as grid_pool,
            tc.tile_pool(name="x1p", bufs=3) as x1_pool,
            tc.tile_pool(name="x2p", bufs=3) as x2_pool,
            tc.tile_pool(name="bp", bufs=2) as b_pool,
            tc.tile_pool(name="outp", bufs=4) as out_pool,
            tc.tile_pool(name="ps", bufs=7, space="PSUM") as psum_pool,
            tc.tile_pool(name="pss", bufs=1, space="PSUM") as psum_s_pool,
            tc.tile_pool(name="dramp", bufs=1, space="DRAM") as dram_pool,
        ):
            w_sb = res_pool.tile([P, KO, O_SH], bf16, tag="w")
            nc.sync.dma_start(w_sb[:], w_in[:])
            g_sb = res_pool.tile([P, KO, O_SH], bf16, tag="g")
            base_sb = res_pool.tile([P, NBT, O_SH], bf16, tag="base")
            acc = res_pool.tile([P, KO], f32, tag="acc")

            # ---- pass 0: G = grid.sum(-1); per-(partition,ko) partial sums
            for ko in range(KO):
                gt = grid_pool.tile([P, GG, O_SH], bf16, tag="gt")
                nc.sync.dma_start(gt[:], g_in[:, ko])
                t1 = grid_pool.tile([P, 4, O_SH], bf16, tag="t1")
                nc.vector.tensor_tensor(t1[:], gt[:, 0:4], gt[:, 4:8], add)
                t2 = grid_pool.tile([P, 2, O_SH], bf16, tag="t2")
                nc.vector.tensor_tensor(t2[:], t1[:, 0:2], t1[:, 2:4], add)
                nc.vector.tensor_tensor(
                    g_sb[:, ko], t2[:, 0], t2[:, 1], add
                )
                nc.vector.tensor_reduce(
                    acc[:, ko : ko + 1],
                    g_sb[:, ko],
                    axis=mybir.AxisListType.X,
                    op=add,
                )

            # scalar grid sum: partition-reduce acc via a tiny ones-matmul,
            # then free-axis reduce of the [1, KO] psum row.
            ones_sb = const_pool.tile([P, 1], f32, tag="ones")
            nc.vector.memset(ones_sb[:], 1.0)
            ps_s = psum_s_pool.tile([1, KO], f32, tag="pss")
            nc.tensor.matmul(ps_s[:], ones_sb[:], acc[:], start=True, stop=True)
            cc_sb = const_pool.tile([1, 8], f32, tag="ccsb")
            nc.vector.memset(cc_sb[:], 0.0)
            nc.vector.tensor_reduce(
                cc_sb[0:1, 0:1], ps_s[:], axis=mybir.AxisListType.X, op=add
            )
            cc_in = dram_pool.tile([1, 8], f32, tag="ccin")
            cc_out = dram_pool.tile([1, 8], f32, tag="ccout")
            nc.sync.dma_start(cc_in[:], cc_sb[:])
            gm_neg = const_pool.tile([P, 1], f32, tag="gmneg")
            if USE_COLLECTIVE:
                nc.gpsimd.collective_compute(
                    "AllReduce",
                    add,
                    replica_groups=[list(range(N_CORES))],
                    ins=[cc_in.opt()],
                    outs=[cc_out.opt()],
                )
                gm_src = cc_out
                # each grid quarter is loaded by R cores -> allreduce sum is
                # R * full-grid sum
                div = R * IN_F * OUT_F * GG
            else:
                # local-shard mean fallback (grid mean is ~N(0, 1e-5); the
                # shard mean is statistically indistinguishable at out tol)
                gm_src = cc_in
                div = IN_F * O_SH * GG
            gm_all = const_pool.tile([P, 1], f32, tag="gmall")
            nc.sync.dma_start(
                gm_all[:], gm_src[0:1, 0:1].to_broadcast((P, 1))
            )
            nc.vector.tensor_scalar_mul(gm_neg[:], gm_all[:], -1.0 / div)

            # ---- pass 1: base_out = x @ W^T  (runs while pass 0 streams)
            for bt in range(NBT):
                xt = x1_pool.tile([P, KO, P], bf16, tag="x1")
                nc.sync.dma_start(xt[:], x_in[:, bt])
                ps = psum_pool.tile([P, O_SH], f32, tag="ps")
                for ko in range(KO):
                    nc.tensor.matmul(
                        ps[:],
                        xt[:, ko],
                        w_sb[:, ko],
                        start=(ko == 0),
                        stop=(ko == KO - 1),
                    )
                nc.scalar.copy(out=base_sb[:, bt], in_=ps[:])

            # ---- pass 2: basis + spline matmul + combine
            for bt in range(NBT):
                xt = x2_pool.tile([P, KO, P], bf16, tag="x2")
                nc.sync.dma_start(xt[:], x_in[:, bt])
                xf = xt.rearrange("p a b -> p (a b)")
                tt = b_pool.tile([P, KO * P], bf16, tag="tt")
                nc.vector.tensor_scalar_add(tt[:], xf, gm_neg[:])
                sq = b_pool.tile([P, KO * P], bf16, tag="sq")
                nc.vector.tensor_tensor(
                    sq[:], tt[:], tt[:], mybir.AluOpType.mult
                )
                bs = b_pool.tile([P, KO, P], bf16, tag="bs")
                nc.scalar.activation(
                    bs.rearrange("p a b -> p (a b)"),
                    sq[:],
                    mybir.ActivationFunctionType.Exp,
                    bias=0.0,
                    scale=-1.0,
                )
                ps = psum_pool.tile([P, O_SH], f32, tag="ps")
                for ko in range(KO):
                    nc.tensor.matmul(
                        ps[:],
                        bs[:, ko],
                        g_sb[:, ko],
                        start=(ko == 0),
                        stop=(ko == KO - 1),
                    )
                ot = out_pool.tile([P, O_SH], f32, tag="ot")
                nc.vector.tensor_tensor(ot[:], ps[:], base_sb[:, bt], add)
                nc.sync.dma_start(out[bt * P : (bt + 1) * P, :], ot[:])

    nc.compile()
    return nc


def _prep_in_maps(x, w, grid):
    in_maps = []
    for core in range(N_CORES):
        r, c = divmod(core, C)
        xs = np.ascontiguousarray(
            x[r * B_SH : (r + 1) * B_SH, :]
            .T.reshape(KO, P, NBT, P)
            .transpose(1, 2, 0, 3)
        ).astype(BF16)
        ws = np.ascontiguousarray(
            w[c * O_SH : (c + 1) * O_SH, :]
            .T.reshape(KO, P, O_SH)
            .transpose(1, 0, 2)
        ).astype(BF16)
        gs = np.ascontiguousarray(
            grid[:, c * O_SH : (c + 1) * O_SH, :]
            .reshape(KO, P, O_SH, GG)
            .transpose(1, 0, 3, 2)
        ).astype(BF16)
        in_maps.append({"xt": xs, "wt": ws, "grid": gs})
    return in_maps


def _gather(results):
    out_full = np.empty((BATCH, OUT_F), np.float32)
    for core in range(N_CORES):
        r, c = divmod(core, C)
        out_full[
            r * B_SH : (r + 1) * B_SH, c * O_SH : (c + 1) * O_SH
        ] = results[core]["out"]
    return out_full


def get_nc():
    global _cached_nc
    if _cached_nc is None:
        _cached_nc = _build_nc()
    return _cached_nc


def run(x, w, grid, **spmd_kwargs):
    nc = get_nc()
    in_maps = _prep_in_maps(x, w, grid)
    res = run_bass_kernel_spmd(
        nc, in_maps, core_ids=list(range(N_CORES)), **spmd_kwargs
    )
    return _gather(res.results), res


def kernel(x, base_weight, grid):
    x = np.asarray(x, dtype=np.float32)
    base_weight = np.asarray(base_weight, dtype=np.float32)
    grid = np.asarray(grid, dtype=np.float32)
    out, _ = run(x, base_weight, grid)
    return out
